# revision 33
# baseline (speedup 1.0000x reference)
"""ChebyNet (K=3, 2 layers) forward on 8 Trainium2 NeuronCores.

Strategy: node sharding, dense-matmul propagation, fp8 DoubleRow PE mode.
Each core owns 1280 padded rows (10000 -> 10240). The sparse propagation
L = -D^-1/2 A D^-1/2 is a dense matmul against the transposed adjacency-count
matrix AT[s, d], SBUF-resident in fp8e4m3 (counts are small ints -> exact).

All four propagation hops run in MatmulPerfMode.DoubleRow: both operands
fp8e4m3, two 128-row k-tiles contracted per PE pass -> 2x effective matmul
throughput (measured 110ns per 256-col DoubleRow matmul vs 2x110ns for the
same work in bf16, LDWEIGHTS fully hidden even when reloaded every matmul).
The propagated features are quantized to fp8 on the DVE before each hop;
the bypass terms (x@(W0-W2), h@(W20-W22)) stay bf16, which keeps the end
to-end relative error ~7e-3 (tolerance 2e-2): per-element fp8 noise ~3.6%
averages out over ~32-edge neighborhoods.

Both layers use linearity of L so each hop propagates the minimum width:

  Layer 1:  h = relu( x(W10-W12) + L( x W11 + L(x 2W12) ) + b1 )
  Layer 2:  out = h(W20-W22) + L( h W21 + L(h 2W22) ) + b2

Pipeline: hop A (single 10-psum wave, psums packed two-per-PSUM-bank with a
single bank-zeroing first matmul) chases the adjacency-chunk DMAs, which are
split across both hardware DGE queues (one queue tops out ~200 GB/s).
Between hops the dis-scaled fp8 features are AllGathered in 3 pieces per
round, emitted as the producing blocks close. All AG staging/reload is
PARTITION-MAJOR: DVE writes land in an SBUF slab, one DMA ships it to the
AG input, and the gathered result reloads as a single wide-line DMA per
piece (a per-kt reload fragments into 256B lines and paces every hop).
Reloads issue from the scalar-engine queue so their AG-semaphore waits
cannot block later staging on the in-order sync queue. Hops C/D run
transposed (stationary = feature pair slab slice, moving = AT) in two
sub-sweeps each, so the next round's pieces / output DMAs stream out early.
"""

import sys

for _p in ("/opt/trn_rl_repo", "/root/.axon_site", "/root/.axon_site/_ro/trn_rl_repo",
           "/root/.axon_site/_ro/pypackages"):
    if _p not in sys.path:
        sys.path.append(_p)

import numpy as np
import ml_dtypes

import concourse.bacc as bacc
import concourse.tile as tile
from concourse import bass, mybir
from concourse.bass_utils import run_bass_kernel_spmd
from concourse.masks import make_identity

# problem constants (hardcoded per harness contract)
N, E, IN, HID, OUT, K = 10000, 320000, 256, 256, 128, 3
CORES = 8
NP = 10240          # padded node count
RPC = NP // CORES   # rows per core = 1280
MB = RPC // 128     # M-blocks per core = 10
KT = NP // 128      # K-tiles = 80
KP = KT // 2        # DoubleRow k-tile pairs = 40
F = IN              # layer-1 prop width = 256
P = 128
CH = 8              # at DMA chunks
KPC = KT // CH      # kts per at chunk = 10
PPC = KPC // 2      # pairs per at chunk = 5
HCH = 16            # xT DMA chunks
HWC = NP // HCH     # xT chunk cols = 640 (5 kts)

FP8 = mybir.dt.float8e4
BF16 = mybir.dt.bfloat16
F32 = mybir.dt.float32
DR = mybir.MatmulPerfMode.DoubleRow

_STATE = {}


def _build():
    nc = bacc.Bacc("TRN2", target_bir_lowering=False, debug=False, num_devices=CORES)

    at_d = nc.dram_tensor("at", [P, KT * RPC], FP8, kind="ExternalInput")
    xoT_d = nc.dram_tensor("xoT", [F, RPC], BF16, kind="ExternalInput")
    # x^T in fp8, chunk-major, IN-halves pair-interleaved for DoubleRow d2
    xT8_d = nc.dram_tensor("xT8", [P, HCH, 2, HWC], FP8, kind="ExternalInput")
    # 2*W1[2] in fp8, IN-halves pair-interleaved (DoubleRow moving operand)
    w12p_d = nc.dram_tensor("w12p", [P, 2, HID], FP8, kind="ExternalInput")
    # own-rows x^T and W1[1] in fp8 pairs: d1 feeds only the fp8-quantized AG
    # path, so it can run in DoubleRow off the pre-R0 critical path
    xoT8p_d = nc.dram_tensor("xoT8p", [P, 2, RPC], FP8, kind="ExternalInput")
    w11p_d = nc.dram_tensor("w11p", [P, 2, HID], FP8, kind="ExternalInput")
    disf_d = nc.dram_tensor("disf", [P, KT], F32, kind="ExternalInput")
    diso_d = nc.dram_tensor("diso", [P, MB], F32, kind="ExternalInput")
    ndiso_d = nc.dram_tensor("ndiso", [P, MB], F32, kind="ExternalInput")
    dd_d = nc.dram_tensor("dd", [P, MB], F32, kind="ExternalInput")
    # w1x = [W1[0]-W1[2], W1[1], 2*W1[2]], w2x likewise for W2
    w1x_d = nc.dram_tensor("w1x", [K, IN, HID], BF16, kind="ExternalInput")
    w2x_d = nc.dram_tensor("w2x", [K, HID, OUT], BF16, kind="ExternalInput")
    b1r_d = nc.dram_tensor("b1r", [P, HID], BF16, kind="ExternalInput")
    b2r_d = nc.dram_tensor("b2r", [P, OUT], BF16, kind="ExternalInput")
    out_d = nc.dram_tensor("outo", [RPC, OUT], F32, kind="ExternalOutput")

    xoT_r = xoT_d.ap().rearrange("(c p) d -> c p d", p=P)
    at_r = at_d.ap().rearrange("p (k d) -> p k d", k=KT)

    with tile.TileContext(nc) as tc:
        with (
            tc.tile_pool(name="res", bufs=1) as res,
            tc.tile_pool(name="wrk", bufs=1) as wrk,
            tc.tile_pool(name="pprop", bufs=5, space="PSUM") as pprop,
            tc.tile_pool(name="pterm", bufs=2, space="PSUM") as pterm,
            tc.tile_pool(name="ptr", bufs=1, space="PSUM") as ptr,
            tc.tile_pool(name="dram", bufs=1, space="DRAM") as dram,
        ):
            # ---- tiny dummy collective first: absorbs the one-time CC
            # bootstrap (~40-55us rendezvous) while the bulk DMAs stream.
            dums = res.tile([P, 16], BF16, name="dums")
            nc.gpsimd.memset(dums[:], 0.0)
            dumi = dram.tile([P, 16], BF16, name="dumi")
            dumo = dram.tile([CORES * P, 16], BF16, name="dumo", addr_space="Shared")
            nc.sync.dma_start(dumi[:], dums[:])
            nc.gpsimd.collective_compute(
                "AllGather", mybir.AluOpType.bypass,
                replica_groups=[list(range(CORES))],
                ins=[dumi[:].opt()], outs=[dumo[:].opt()],
            )

            # ---- small loads ----
            xoT_t = []
            for c in range(2):
                t = res.tile([P, RPC], BF16, tag="xoTsT", bufs=2, name=f"xoT{c}")
                nc.sync.dma_start(t[:], xoT_r[c])
                xoT_t.append(t)
            w1t0 = [None, None]
            for c in range(2):
                t = res.tile([P, HID], BF16, tag="wh", bufs=6, name=f"w1_0_{c}")
                nc.sync.dma_start(t[:], w1x_d[0, c * P:(c + 1) * P, :])
                w1t0[c] = t
            w12p = res.tile([P, 2, HID], FP8, name="w12p")
            nc.sync.dma_start(w12p[:], w12p_d[:])
            xoT8p = res.tile([P, 2, RPC], FP8, name="xoT8p")
            nc.sync.dma_start(xoT8p[:], xoT8p_d[:])
            w11p = res.tile([P, 2, HID], FP8, name="w11p")
            nc.sync.dma_start(w11p[:], w11p_d[:])
            w2t = [[None, None] for _ in range(K)]
            for k in range(K):
                for c in range(2):
                    t = res.tile([P, OUT], BF16, tag=f"w2_{k}_{c}", name=f"w2_{k}_{c}")
                    nc.sync.dma_start(t[:], w2x_d[k, c * P:(c + 1) * P, :])
                    w2t[k][c] = t
            diso = res.tile([P, MB], F32, name="diso")
            nc.sync.dma_start(diso[:], diso_d[:])
            ndiso = res.tile([P, MB], F32, name="ndiso")
            nc.sync.dma_start(ndiso[:], ndiso_d[:])
            dd = res.tile([P, MB], F32, name="dd")
            nc.sync.dma_start(dd[:], dd_d[:])
            b1r = res.tile([P, HID], BF16, name="b1r")
            nc.sync.dma_start(b1r[:], b1r_d[:])
            b2r = res.tile([P, OUT], BF16, name="b2r")
            nc.sync.dma_start(b2r[:], b2r_d[:])
            disf = res.tile([P, KT], F32, name="disf")
            nc.sync.dma_start(disf[:], disf_d[:])

            ident = res.tile([P, P], F32, name="ident")
            make_identity(nc, ident[:])
            idb = res.tile([P, P], BF16, name="idb")
            nc.vector.tensor_copy(idb[:], ident[:])

            # ---- bulk DMAs, interleaved in chunk groups so arrival is
            # ordered (xT for d2 first, then the at sub-chunks the matmuls of
            # that group consume). The at sub-DMAs alternate between the two
            # hardware DGE queues (sync=SP, scalar=Act): a single queue tops
            # out ~200 GB/s and the 13 MB at stream would pace the kernel.
            atc = []
            xT8c_t = [None] * HCH
            for c8 in range(CH):
                t = res.tile([P, KPC, RPC], FP8, name=f"atc{c8}")
                atc.append(t)
                for hc in (2 * c8, 2 * c8 + 1):
                    xt = wrk.tile([P, 2, HWC], FP8, tag="xTc", bufs=4,
                                  name=f"xTc{hc}")
                    # scalar queue: lands ahead of the at stream so the d2
                    # chase starts as soon as w12p arrives
                    nc.scalar.dma_start(xt[:], xT8_d[:, hc, :, :])
                    xT8c_t[hc] = xt
                for s0 in range(0, KPC, 2):
                    eng = nc.scalar if (c8 * PPC + s0 // 2) % 2 == 0 else nc.sync
                    eng.dma_start(t[:, s0:s0 + 2, :],
                                  at_r[:, c8 * KPC + s0:c8 * KPC + s0 + 2, :])

            def at_pair(kp, lo, w):
                c8, r = divmod(kp, PPC)
                return atc[c8][:, 2 * r:2 * r + 2, lo:lo + w]

            # persistent per-block tensors (bf16; tags shared across layers)
            d1z_t = [res.tile([P, F], BF16, tag="dz", bufs=MB, name=f"d1_{m}")
                     for m in range(MB)]
            hw_t = [res.tile([P, F], BF16, tag="ehw", bufs=MB, name=f"e0_{m}")
                    for m in range(MB)]  # holds e0 now, hw later (slot reuse)
            e0_t = hw_t

            # uA pair tiles (fp8): hop A input (dis*d2), computed locally.
            uA = [res.tile([P, 2, F], FP8, tag=f"uA{kp}", name=f"uA{kp}")
                  for kp in range(KP)]

            def mm6(psum_ap, lhsTs, rhs_pair):
                nc.tensor.matmul(psum_ap, lhsTs[0], rhs_pair[0][:], start=True, stop=False)
                nc.tensor.matmul(psum_ap, lhsTs[1], rhs_pair[1][:], start=False, stop=True)

            # ---- d1 = x@W11 (DoubleRow fp8: feeds only the fp8 AG path) ----
            xoT_sl = [[xoT_t[c][:, m * P:(m + 1) * P] for c in range(2)] for m in range(MB)]
            for mb in range(MB):
                dp = pterm.tile([P, F], F32, tag="tp", name=f"d1p_{mb}")
                nc.tensor.matmul(dp[:], xoT8p[:, :, mb * P:(mb + 1) * P],
                                 w11p[:], start=True, stop=True, perf_mode=DR)
                nc.vector.tensor_scalar_mul(d1z_t[mb][:], dp[:], diso[:, mb:mb + 1])

            # ---- hop A: single wave, all 10 dst-blocks accumulate while the
            # at/xT chunk DMAs stream in (10 psums packed 2-per-PSUM-bank).
            # uA (fp8 pair tiles) for each chunk's kts is computed just ahead
            # of the matmuls that consume it.
            ppA_bank = [pprop.tile([P, 2 * F], F32, tag="pp", name=f"ppA_{b}")
                        for b in range(5)]
            ppA = [ppA_bank[mb // 2][:, (mb % 2) * F:(mb % 2 + 1) * F]
                   for mb in range(MB)]
            for c8 in range(CH):
                for hc in (2 * c8, 2 * c8 + 1):
                    for m5 in range(5):
                        kt = hc * 5 + m5
                        dp = pterm.tile([P, F], F32, tag="tp", name=f"d2f_{kt}")
                        nc.tensor.matmul(
                            dp[:], xT8c_t[hc][:, :, m5 * P:(m5 + 1) * P],
                            w12p[:], start=True, stop=True, perf_mode=DR,
                        )
                        nc.vector.tensor_scalar_mul(uA[kt // 2][:, kt % 2, :], dp[:],
                                                    disf[:, kt:kt + 1])
                for r in range(PPC):
                    kp = c8 * PPC + r
                    for mb in range(MB):
                        # start=True zeroes the WHOLE 2KB psum bank, so only
                        # the first matmul into each shared bank may carry it
                        nc.tensor.matmul(
                            ppA[mb], at_pair(kp, mb * P, P), uA[kp][:],
                            start=(kp == 0 and mb % 2 == 0),
                            stop=(kp == KP - 1), perf_mode=DR,
                            skip_group_check=True,
                        )

            # ---- AG plumbing (fp8), 3 pieces per round, pair-aligned.
            # Everything stays PARTITION-MAJOR end to end: DVE staging writes
            # land in an SBUF slab, ONE DMA ships the slab to the AG input,
            # and the gathered output reloads as ONE DMA per piece with
            # len*w-byte lines (the naive per-kt reload fragments into 256B
            # lines and trickles at ~15GB/s, pacing every hop).
            PCM = [(0, 1, 2, 3), (4, 5, 6, 7), (8, 9)]      # mb groups
            PCP = [(0, 1), (2, 3), (4,)]                    # pair idx groups
            LENP = [len(ms) for ms in PCM]

            def mk_round(tag, w):
                stg = [wrk.tile([P, LENP[i] * w], FP8, tag=f"stg{tag}_{i}",
                                name=f"stg{tag}_{i}") for i in range(3)]
                agi = [dram.tile([P, LENP[i] * w], FP8, name=f"agi{tag}_{i}")
                       for i in range(3)]
                ago = [dram.tile([CORES * P, LENP[i] * w], FP8,
                                 name=f"ago{tag}_{i}", addr_space="Shared")
                       for i in range(3)]
                slab = [res.tile([P, CORES, LENP[i] * w], FP8,
                                 name=f"slab{tag}_{i}") for i in range(3)]
                return stg, agi, ago, slab

            stg0, agi0, ago0, slab0 = mk_round("0", F)
            stg1, agi1, ago1, slab1 = mk_round("1", OUT)
            stg2, agi2, ago2, slab2 = mk_round("2", OUT)

            def piece_of(mb):
                i = mb // 4
                return i, mb - (0, 4, 8)[i]

            def emit_round(stg_i, agi_i, ago_i, slab_i):
                nc.sync.dma_start(agi_i[:], stg_i[:])
                nc.gpsimd.collective_compute(
                    "AllGather", mybir.AluOpType.bypass,
                    replica_groups=[list(range(CORES))],
                    ins=[agi_i[:].opt()], outs=[ago_i[:].opt()],
                )
                # slab reload split per source core, in kp-consumption order,
                # so the first pairs are usable right after the CC completes.
                # Scalar queue so the AG-semaphore wait can't block staging
                # DMAs behind it on the in-order sync queue.
                for c8 in range(CORES):
                    nc.scalar.dma_start(
                        slab_i[:, c8:c8 + 1, :].rearrange("p c n -> p (c n)"),
                        ago_i[c8 * P:(c8 + 1) * P, :])

            def u_of(slabs, w):
                def f(kp):
                    c8, mp = divmod(kp, PPC)
                    i = 0 if mp < 2 else (1 if mp < 4 else 2)
                    j = mp - (0, 2, 4)[i]
                    return slabs[i][:, c8:c8 + 1, 2 * j * w:(2 * j + 2) * w] \
                        .rearrange("p c (k n) -> p (c k) n", k=2)
                return f

            uB0 = u_of(slab0, F)    # hop B moving operand (s1)
            uA1 = u_of(slab1, OUT)  # hop C stationary (dis*z2)
            uB2 = u_of(slab2, OUT)  # hop D stationary (sD)

            def stage_s1(mb, pp_ap):
                i, idx = piece_of(mb)
                t = wrk.tile([P, F], BF16, tag="st", bufs=2, name=f"stA_{mb}")
                nc.vector.tensor_scalar_mul(t[:], pp_ap, dd[:, mb:mb + 1])
                nc.vector.tensor_add(stg0[i][:, idx * F:(idx + 1) * F],
                                     t[:], d1z_t[mb][:])

            for mb in range(MB):
                stage_s1(mb, ppA[mb])
                if mb in (3, 7, 9):
                    i = piece_of(mb)[0]
                    emit_round(stg0[i], agi0[i], ago0[i], slab0[i])

            # ---- e0 = x@(W10-W12): bf16 bypass term, deliberately computed
            # during the round-0 AG wait. Its psums come from the hop-A psum
            # tag, so the pool-rotation WAR dependency pins it AFTER hop A's
            # staging (the scheduler would otherwise hoist it pre-R0, putting
            # its slow bf16-stationary LDWEIGHTS back on the critical path).
            for mb in range(MB):
                e0p = pprop.tile([P, F], F32, tag="pp", name=f"e0p_{mb}")
                mm6(e0p[:], xoT_sl[mb], w1t0)
                nc.vector.tensor_copy(e0_t[mb][:], e0p[:])

            # ---- hop B + layer-2 feature matmuls, software-pipelined ----
            # kp consumption order matches round-0 piece arrival
            orderB = [c8 * PPC + mp for ms in PCP for mp in ms
                      for c8 in range(CORES)]

            pp_b = [None] * MB

            def post_B(m):
                h = wrk.tile([P, F], BF16, tag="h", bufs=2, name=f"h_{m}")
                nc.vector.tensor_scalar_mul(h[:], pp_b[m][:], ndiso[:, m:m + 1])
                nc.vector.tensor_add(h[:], h[:], e0_t[m][:])
                nc.vector.tensor_add(h[:], h[:], b1r[:])
                nc.vector.tensor_scalar_max(h[:], h[:], 0.0)
                hT = []
                for c in range(2):
                    tps = ptr.tile([P, P], BF16, tag="tr", name=f"hTp_{m}_{c}")
                    nc.tensor.transpose(tps[:], h[:, c * P:(c + 1) * P], idb[:])
                    tb = res.tile([P, HID], BF16, tag="wh", bufs=6, name=f"hTs_{m}_{c}")
                    nc.vector.tensor_copy(tb[:, :P], tps[:])
                    hT.append(tb[:, :P])
                # all three h@W2k products in one psum, hT stationary loaded
                # once per c-half (3 consecutive matmuls share the LDWEIGHTS)
                zall = pterm.tile([P, 3 * OUT], F32, tag="tp", name=f"zall_{m}")
                for c in range(2):
                    for j, k in enumerate((1, 2, 0)):  # z1 | z2 | hw
                        # single bank-zero on the first matmul (see hop A note)
                        nc.tensor.matmul(zall[:, j * OUT:(j + 1) * OUT],
                                         hT[c], w2t[k][c][:],
                                         start=(c == 0 and j == 0), stop=(c == 1),
                                         skip_group_check=True)
                nc.vector.tensor_scalar_mul(d1z_t[m][:, :OUT], zall[:, :OUT],
                                            diso[:, m:m + 1])
                i, idx = piece_of(m)
                nc.vector.tensor_scalar_mul(stg1[i][:, idx * OUT:(idx + 1) * OUT],
                                            zall[:, OUT:2 * OUT], diso[:, m:m + 1])
                nc.vector.tensor_copy(hw_t[m][:, :OUT], zall[:, 2 * OUT:3 * OUT])
                if m in (3, 7, 9):
                    emit_round(stg1[i], agi1[i], ago1[i], slab1[i])

            for mb in range(5):
                pp_b[mb] = pprop.tile([P, F], F32, tag="pp", name=f"ppb_{mb}")
            for kp in orderB:
                for mb in range(5):
                    nc.tensor.matmul(
                        pp_b[mb][:], at_pair(kp, mb * P, P), uB0(kp),
                        start=(kp == orderB[0]), stop=(kp == orderB[-1]),
                        perf_mode=DR,
                    )
            for mb in range(5):
                post_B(mb)
            for mb in range(5, MB + 1):
                if mb < MB:
                    pp_b[mb] = pprop.tile([P, F], F32, tag="pp", name=f"ppb_{mb}")
                    for kp in orderB:
                        nc.tensor.matmul(
                            pp_b[mb][:], at_pair(kp, mb * P, P), uB0(kp),
                            start=(kp == orderB[0]), stop=(kp == orderB[-1]),
                            perf_mode=DR,
                        )
                if mb > 5:
                    post_B(mb - 1)

            # ---- hops C and D: transposed (stationary = u pair tile, moving
            # = AT), psum j covers dst cols [j*256, (j+1)*256) = mbs (2j,2j+1).
            # Hop C runs in two sub-sweeps so R2 pieces stream out early.
            # kp order matches the 3-piece arrival of the previous round.
            orderCD = orderB

            ppc = [None] * PPC

            def sweep_T(u_fn, js, order, tagn):
                for j in js:
                    ppc[j] = pprop.tile([P, 2 * P], F32, tag="pp",
                                        name=f"{tagn}_{j}")
                for kp in order:
                    for j in js:
                        nc.tensor.matmul(
                            ppc[j][:], u_fn(kp),
                            at_pair(kp, j * 2 * P, 2 * P),
                            start=(kp == order[0]), stop=(kp == order[-1]),
                            perf_mode=DR,
                        )

            def post_C(j):
                sT = res.tile([P, 2 * P], F32, tag="xoTsT", bufs=2, name=f"sTc_{j}")
                nc.vector.tensor_copy(sT[:], ppc[j][:])
                for i in range(2):
                    mb = 2 * j + i
                    tps = ptr.tile([P, P], F32, tag="tr", name=f"trC_{mb}")
                    nc.tensor.transpose(tps[:], sT[:, i * P:(i + 1) * P], ident[:])
                    t = wrk.tile([P, F], BF16, tag="st", bufs=2, name=f"stC_{mb}")
                    nc.vector.tensor_scalar_mul(t[:, :OUT], tps[:], dd[:, mb:mb + 1])
                    i2, idx = piece_of(mb)
                    nc.vector.tensor_add(stg2[i2][:, idx * OUT:(idx + 1) * OUT],
                                         t[:, :OUT], d1z_t[mb][:, :OUT])
                    if mb in (3, 7, 9):
                        emit_round(stg2[i2], agi2[i2], ago2[i2], slab2[i2])

            # one sub-sweep per psum so each post_C (and its R2 piece) overlaps
            # the next sub-sweep's matmuls
            for j in range(PPC):
                sweep_T(uA1, (j,), orderCD, f"ppc{j}")
                post_C(j)

            def post_D(j):
                sT = res.tile([P, 2 * P], F32, tag="xoTsT", bufs=2, name=f"sTd_{j}")
                nc.vector.tensor_copy(sT[:], ppc[j][:])
                for i in range(2):
                    mb = 2 * j + i
                    tps = ptr.tile([P, P], F32, tag="tr", name=f"trD_{mb}")
                    nc.tensor.transpose(tps[:], sT[:, i * P:(i + 1) * P], ident[:])
                    oacc = wrk.tile([P, OUT], F32, tag="sf", bufs=2, name=f"oacc_{mb}")
                    nc.vector.tensor_scalar_mul(oacc[:], tps[:], ndiso[:, mb:mb + 1])
                    nc.vector.tensor_add(oacc[:], oacc[:], hw_t[mb][:, :OUT])
                    nc.vector.tensor_add(oacc[:], oacc[:], b2r[:])
                    nc.sync.dma_start(out_d[mb * P:(mb + 1) * P, :], oacc[:])

            # hop D in two sub-sweeps so the first output rows DMA out while
            # the second sub-sweep still accumulates
            # one sub-sweep per psum so each post_D (output rows) overlaps the
            # next sub-sweep's matmuls
            for j in range(PPC):
                sweep_T(uB2, (j,), orderCD, f"ppd{j}")
                post_D(j)

    nc.compile()
    return nc


def _prepare_inputs(x, edge, W1, b1, W2, b2):
    x = np.asarray(x, np.float32)
    edge = np.asarray(edge)
    W1 = np.asarray(W1, np.float32)
    b1 = np.asarray(b1, np.float32)
    W2 = np.asarray(W2, np.float32)
    b2 = np.asarray(b2, np.float32)
    src = edge[0].astype(np.int64)
    dst = edge[1].astype(np.int64)

    deg = np.bincount(dst, minlength=N).astype(np.float32)
    dis = np.where(deg > 0, 1.0 / np.sqrt(np.maximum(deg, 1.0)), 0.0).astype(np.float32)

    # dense transposed adjacency counts AT[s, d]
    flat = src * NP + dst
    uniq, cnt = np.unique(flat, return_counts=True)
    at8 = np.zeros(NP * NP, dtype=ml_dtypes.float8_e4m3)
    at8[uniq] = cnt.astype(ml_dtypes.float8_e4m3)
    at8 = at8.reshape(NP, NP)

    dis_pad = np.zeros(NP, np.float32)
    dis_pad[:N] = dis
    x_pad = np.zeros((NP, F), np.float32)
    x_pad[:N] = x

    w1x = np.stack([W1[0] - W1[2], W1[1], 2.0 * W1[2]]).astype(ml_dtypes.bfloat16)
    w2x = np.stack([W2[0] - W2[2], W2[1], 2.0 * W2[2]]).astype(ml_dtypes.bfloat16)
    b1r = np.broadcast_to(b1, (P, HID)).astype(ml_dtypes.bfloat16).copy()
    b2r = np.broadcast_to(b2, (P, OUT)).astype(ml_dtypes.bfloat16).copy()

    # x^T fp8, chunk-major with IN-halves pair-interleaved: [P, HCH, 2, HWC]
    xT8 = np.ascontiguousarray(
        x_pad.T.astype(ml_dtypes.float8_e4m3)
        .reshape(2, P, HCH, HWC).transpose(1, 2, 0, 3))
    w12p = np.ascontiguousarray(
        (2.0 * W1[2]).astype(ml_dtypes.float8_e4m3)
        .reshape(2, P, HID).transpose(1, 0, 2))
    w11p = np.ascontiguousarray(
        W1[1].astype(ml_dtypes.float8_e4m3)
        .reshape(2, P, HID).transpose(1, 0, 2))
    disf_h = np.ascontiguousarray(dis_pad.reshape(KT, P).T)
    in_maps = []
    for c in range(CORES):
        rows = slice(c * RPC, (c + 1) * RPC)
        dv = dis_pad[rows]
        atc = np.ascontiguousarray(
            at8[:, rows].reshape(KT, P, RPC).transpose(1, 0, 2).reshape(P, KT * RPC))
        m = {
            "at": atc,
            "xoT": np.ascontiguousarray(x_pad[rows].T).astype(ml_dtypes.bfloat16),
            "xoT8p": np.ascontiguousarray(
                x_pad[rows].T.astype(ml_dtypes.float8_e4m3)
                .reshape(2, P, RPC).transpose(1, 0, 2)),
            "xT8": xT8,
            "w12p": w12p,
            "w11p": w11p,
            "disf": disf_h,
            "diso": np.ascontiguousarray(dv.reshape(MB, P).T),
            "ndiso": np.ascontiguousarray((-dv).reshape(MB, P).T),
            "dd": np.ascontiguousarray((-dv * dv).reshape(MB, P).T),
            "w1x": w1x,
            "w2x": w2x,
            "b1r": b1r,
            "b2r": b2r,
        }
        in_maps.append(m)
    return in_maps


def _run(in_maps, trace=False, **kw):
    if "nc" not in _STATE:
        _STATE["nc"] = _build()
    r = run_bass_kernel_spmd(_STATE["nc"], in_maps, core_ids=list(range(CORES)),
                             trace=trace, **kw)
    out = np.concatenate([r.results[c]["outo"] for c in range(CORES)], axis=0)
    return out[:N], r


def kernel(**inputs) -> np.ndarray:
    in_maps = _prepare_inputs(**inputs)
    out, _ = _run(in_maps)
    return out


# revision 34
# speedup vs baseline: 1.2286x; 1.2286x over previous
"""ChebyNet (K=3, 2 layers) forward on 8 Trainium2 NeuronCores.

Strategy: node sharding, dense-matmul propagation, fp8 DoubleRow PE mode.
Each core owns 1280 padded rows (10000 -> 10240). The sparse propagation
L = -D^-1/2 A D^-1/2 is a dense matmul against the transposed adjacency-count
matrix AT[s, d], SBUF-resident in fp8e4m3 (counts are small ints -> exact).

All four propagation hops run in MatmulPerfMode.DoubleRow: both operands
fp8e4m3, two 128-row k-tiles contracted per PE pass -> 2x effective matmul
throughput (measured 110ns per 256-col DoubleRow matmul vs 2x110ns for the
same work in bf16, LDWEIGHTS fully hidden even when reloaded every matmul).
The propagated features are quantized to fp8 on the DVE before each hop;
the bypass terms (x@(W0-W2), h@(W20-W22)) stay bf16, which keeps the end
to-end relative error ~7e-3 (tolerance 2e-2): per-element fp8 noise ~3.6%
averages out over ~32-edge neighborhoods.

Both layers use linearity of L so each hop propagates the minimum width:

  Layer 1:  h = relu( x(W10-W12) + L( x W11 + L(x 2W12) ) + b1 )
  Layer 2:  out = h(W20-W22) + L( h W21 + L(h 2W22) ) + b2

Pipeline: hop A (single 10-psum wave, psums packed two-per-PSUM-bank with a
single bank-zeroing first matmul) chases the adjacency-chunk DMAs, which are
split across both hardware DGE queues (one queue tops out ~200 GB/s).
Between hops the dis-scaled fp8 features are AllGathered in 3 pieces per
round, emitted as the producing blocks close. All AG staging/reload is
PARTITION-MAJOR: DVE writes land in an SBUF slab, one DMA ships it to the
AG input, and the gathered result reloads as a single wide-line DMA per
piece (a per-kt reload fragments into 256B lines and paces every hop).
Reloads issue from the scalar-engine queue so their AG-semaphore waits
cannot block later staging on the in-order sync queue. Hops C/D run
transposed (stationary = feature pair slab slice, moving = AT) in two
sub-sweeps each, so the next round's pieces / output DMAs stream out early.
"""

import sys

for _p in ("/opt/trn_rl_repo", "/root/.axon_site", "/root/.axon_site/_ro/trn_rl_repo",
           "/root/.axon_site/_ro/pypackages"):
    if _p not in sys.path:
        sys.path.append(_p)

import numpy as np
import ml_dtypes

import concourse.bacc as bacc
import concourse.tile as tile
from concourse import bass, mybir
from concourse.bass_utils import run_bass_kernel_spmd
from concourse.masks import make_identity

# problem constants (hardcoded per harness contract)
N, E, IN, HID, OUT, K = 10000, 320000, 256, 256, 128, 3
CORES = 8
NP = 10240          # padded node count
RPC = NP // CORES   # rows per core = 1280
MB = RPC // 128     # M-blocks per core = 10
KT = NP // 128      # K-tiles = 80
KP = KT // 2        # DoubleRow k-tile pairs = 40
F = IN              # layer-1 prop width = 256
P = 128
CH = 8              # at DMA chunks
KPC = KT // CH      # kts per at chunk = 10
PPC = KPC // 2      # pairs per at chunk = 5
HCH = 16            # xT DMA chunks
HWC = NP // HCH     # xT chunk cols = 640 (5 kts)

FP8 = mybir.dt.float8e4
BF16 = mybir.dt.bfloat16
F32 = mybir.dt.float32
DR = mybir.MatmulPerfMode.DoubleRow

_STATE = {}


def _build():
    nc = bacc.Bacc("TRN2", target_bir_lowering=False, debug=False, num_devices=CORES)

    at_d = nc.dram_tensor("at", [P, KT * RPC], FP8, kind="ExternalInput")
    xoT_d = nc.dram_tensor("xoT", [F, RPC], BF16, kind="ExternalInput")
    # x^T in fp8, chunk-major, IN-halves pair-interleaved for DoubleRow d2
    xT8_d = nc.dram_tensor("xT8", [P, HCH, 2, HWC], FP8, kind="ExternalInput")
    # 2*W1[2] in fp8, IN-halves pair-interleaved (DoubleRow moving operand)
    w12p_d = nc.dram_tensor("w12p", [P, 2, HID], FP8, kind="ExternalInput")
    disf_d = nc.dram_tensor("disf", [P, KT], F32, kind="ExternalInput")
    diso_d = nc.dram_tensor("diso", [P, MB], F32, kind="ExternalInput")
    ndiso_d = nc.dram_tensor("ndiso", [P, MB], F32, kind="ExternalInput")
    dd_d = nc.dram_tensor("dd", [P, MB], F32, kind="ExternalInput")
    # w1x = [W1[0]-W1[2], W1[1], 2*W1[2]], w2x likewise for W2
    w1x_d = nc.dram_tensor("w1x", [K, IN, HID], BF16, kind="ExternalInput")
    w2x_d = nc.dram_tensor("w2x", [K, HID, OUT], BF16, kind="ExternalInput")
    b1r_d = nc.dram_tensor("b1r", [P, HID], BF16, kind="ExternalInput")
    b2r_d = nc.dram_tensor("b2r", [P, OUT], BF16, kind="ExternalInput")
    out_d = nc.dram_tensor("outo", [RPC, OUT], F32, kind="ExternalOutput")

    xoT_r = xoT_d.ap().rearrange("(c p) d -> c p d", p=P)
    at_r = at_d.ap().rearrange("p (k d) -> p k d", k=KT)

    with tile.TileContext(nc) as tc:
        with (
            tc.tile_pool(name="res", bufs=1) as res,
            tc.tile_pool(name="wrk", bufs=1) as wrk,
            tc.tile_pool(name="pprop", bufs=5, space="PSUM") as pprop,
            tc.tile_pool(name="pterm", bufs=2, space="PSUM") as pterm,
            tc.tile_pool(name="ptr", bufs=1, space="PSUM") as ptr,
            tc.tile_pool(name="dram", bufs=1, space="DRAM") as dram,
        ):
            # ---- tiny dummy collective first: absorbs the one-time CC
            # bootstrap (~40-55us rendezvous) while the bulk DMAs stream.
            dums = res.tile([P, 16], BF16, name="dums")
            nc.gpsimd.memset(dums[:], 0.0)
            dumi = dram.tile([P, 16], BF16, name="dumi")
            dumo = dram.tile([CORES * P, 16], BF16, name="dumo", addr_space="Shared")
            nc.sync.dma_start(dumi[:], dums[:])
            nc.gpsimd.collective_compute(
                "AllGather", mybir.AluOpType.bypass,
                replica_groups=[list(range(CORES))],
                ins=[dumi[:].opt()], outs=[dumo[:].opt()],
            )

            # ---- small loads ----
            xoT_t = []
            for c in range(2):
                t = res.tile([P, RPC], BF16, tag="xoTsT", bufs=2, name=f"xoT{c}")
                nc.sync.dma_start(t[:], xoT_r[c])
                xoT_t.append(t)
            w1t = [[None, None] for _ in range(2)]
            for k in range(2):
                for c in range(2):
                    t = res.tile([P, HID], BF16, tag="wh", bufs=6, name=f"w1_{k}_{c}")
                    nc.sync.dma_start(t[:], w1x_d[k, c * P:(c + 1) * P, :])
                    w1t[k][c] = t
            w12p = res.tile([P, 2, HID], FP8, name="w12p")
            nc.sync.dma_start(w12p[:], w12p_d[:])
            w2t = [[None, None] for _ in range(K)]
            for k in range(K):
                for c in range(2):
                    t = res.tile([P, OUT], BF16, tag=f"w2_{k}_{c}", name=f"w2_{k}_{c}")
                    nc.sync.dma_start(t[:], w2x_d[k, c * P:(c + 1) * P, :])
                    w2t[k][c] = t
            diso = res.tile([P, MB], F32, name="diso")
            nc.sync.dma_start(diso[:], diso_d[:])
            ndiso = res.tile([P, MB], F32, name="ndiso")
            nc.sync.dma_start(ndiso[:], ndiso_d[:])
            dd = res.tile([P, MB], F32, name="dd")
            nc.sync.dma_start(dd[:], dd_d[:])
            b1r = res.tile([P, HID], BF16, name="b1r")
            nc.sync.dma_start(b1r[:], b1r_d[:])
            b2r = res.tile([P, OUT], BF16, name="b2r")
            nc.sync.dma_start(b2r[:], b2r_d[:])
            disf = res.tile([P, KT], F32, name="disf")
            nc.sync.dma_start(disf[:], disf_d[:])

            ident = res.tile([P, P], F32, name="ident")
            make_identity(nc, ident[:])
            idb = res.tile([P, P], BF16, name="idb")
            nc.vector.tensor_copy(idb[:], ident[:])

            # ---- bulk DMAs, interleaved in chunk groups so arrival is
            # ordered (xT for d2 first, then the at sub-chunks the matmuls of
            # that group consume). The at sub-DMAs alternate between the two
            # hardware DGE queues (sync=SP, scalar=Act): a single queue tops
            # out ~200 GB/s and the 13 MB at stream would pace the kernel.
            atc = []
            xT8c_t = [None] * HCH
            for c8 in range(CH):
                t = res.tile([P, KPC, RPC], FP8, name=f"atc{c8}")
                atc.append(t)
                for hc in (2 * c8, 2 * c8 + 1):
                    xt = wrk.tile([P, 2, HWC], FP8, tag="xTc", bufs=4,
                                  name=f"xTc{hc}")
                    # scalar queue: lands ahead of the at stream so the d2
                    # chase starts as soon as w12p arrives
                    nc.scalar.dma_start(xt[:], xT8_d[:, hc, :, :])
                    xT8c_t[hc] = xt
                for s0 in range(0, KPC, 2):
                    eng = nc.scalar if (c8 * PPC + s0 // 2) % 2 == 0 else nc.sync
                    eng.dma_start(t[:, s0:s0 + 2, :],
                                  at_r[:, c8 * KPC + s0:c8 * KPC + s0 + 2, :])

            def at_pair(kp, lo, w):
                c8, r = divmod(kp, PPC)
                return atc[c8][:, 2 * r:2 * r + 2, lo:lo + w]

            # persistent per-block tensors (bf16; tags shared across layers)
            d1z_t = [res.tile([P, F], BF16, tag="dz", bufs=MB, name=f"d1_{m}")
                     for m in range(MB)]
            hw_t = [res.tile([P, F], BF16, tag="ehw", bufs=MB, name=f"e0_{m}")
                    for m in range(MB)]  # holds e0 now, hw later (slot reuse)
            e0_t = hw_t

            # uA pair tiles (fp8): hop A input (dis*d2), computed locally.
            uA = [res.tile([P, 2, F], FP8, tag=f"uA{kp}", name=f"uA{kp}")
                  for kp in range(KP)]

            def mm6(psum_ap, lhsTs, rhs_pair):
                nc.tensor.matmul(psum_ap, lhsTs[0], rhs_pair[0][:], start=True, stop=False)
                nc.tensor.matmul(psum_ap, lhsTs[1], rhs_pair[1][:], start=False, stop=True)

            # ---- d1 = x@W11, e0 = x@(W10-W12) from own-rows x^T.
            # Both into one psum bank with the bf16 xoT stationary loaded once
            # per c-half (bf16 LDWEIGHTS is slow, ~360ns, so reuse it).
            xoT_sl = [[xoT_t[c][:, m * P:(m + 1) * P] for c in range(2)] for m in range(MB)]
            for mb in range(MB):
                de = pterm.tile([P, 2 * F], F32, tag="tp", name=f"de_{mb}")
                for c in range(2):
                    for r, w1 in ((0, w1t[1]), (1, w1t[0])):  # d1 | e0
                        nc.tensor.matmul(de[:, r * F:(r + 1) * F],
                                         xoT_sl[mb][c], w1[c][:],
                                         start=(c == 0 and r == 0), stop=(c == 1),
                                         skip_group_check=True)
                nc.vector.tensor_scalar_mul(d1z_t[mb][:], de[:, :F], diso[:, mb:mb + 1])
                nc.vector.tensor_copy(e0_t[mb][:], de[:, F:2 * F])

            # ---- hop A: single wave, all 10 dst-blocks accumulate while the
            # at/xT chunk DMAs stream in (10 psums packed 2-per-PSUM-bank).
            # uA (fp8 pair tiles) for each chunk's kts is computed just ahead
            # of the matmuls that consume it.
            ppA_bank = [pprop.tile([P, 2 * F], F32, tag="pp", name=f"ppA_{b}")
                        for b in range(5)]
            ppA = [ppA_bank[mb // 2][:, (mb % 2) * F:(mb % 2 + 1) * F]
                   for mb in range(MB)]
            for c8 in range(CH):
                for hc in (2 * c8, 2 * c8 + 1):
                    for m5 in range(5):
                        kt = hc * 5 + m5
                        dp = pterm.tile([P, F], F32, tag="tp", name=f"d2f_{kt}")
                        nc.tensor.matmul(
                            dp[:], xT8c_t[hc][:, :, m5 * P:(m5 + 1) * P],
                            w12p[:], start=True, stop=True, perf_mode=DR,
                        )
                        nc.vector.tensor_scalar_mul(uA[kt // 2][:, kt % 2, :], dp[:],
                                                    disf[:, kt:kt + 1])
                for r in range(PPC):
                    kp = c8 * PPC + r
                    for mb in range(MB):
                        # start=True zeroes the WHOLE 2KB psum bank, so only
                        # the first matmul into each shared bank may carry it
                        nc.tensor.matmul(
                            ppA[mb], at_pair(kp, mb * P, P), uA[kp][:],
                            start=(kp == 0 and mb % 2 == 0),
                            stop=(kp == KP - 1), perf_mode=DR,
                            skip_group_check=True,
                        )

            # ---- AG plumbing (fp8), 3 pieces per round, pair-aligned.
            # Everything stays PARTITION-MAJOR end to end: DVE staging writes
            # land in an SBUF slab, ONE DMA ships the slab to the AG input,
            # and the gathered output reloads as ONE DMA per piece with
            # len*w-byte lines (the naive per-kt reload fragments into 256B
            # lines and trickles at ~15GB/s, pacing every hop).
            PCM = [(0, 1, 2, 3), (4, 5, 6, 7), (8, 9)]      # mb groups
            PCP = [(0, 1), (2, 3), (4,)]                    # pair idx groups
            LENP = [len(ms) for ms in PCM]

            def mk_round(tag, w):
                stg = [wrk.tile([P, LENP[i] * w], FP8, tag=f"stg{tag}_{i}",
                                name=f"stg{tag}_{i}") for i in range(3)]
                agi = [dram.tile([P, LENP[i] * w], FP8, name=f"agi{tag}_{i}")
                       for i in range(3)]
                ago = [dram.tile([CORES * P, LENP[i] * w], FP8,
                                 name=f"ago{tag}_{i}", addr_space="Shared")
                       for i in range(3)]
                slab = [res.tile([P, CORES, LENP[i] * w], FP8,
                                 name=f"slab{tag}_{i}") for i in range(3)]
                return stg, agi, ago, slab

            stg0, agi0, ago0, slab0 = mk_round("0", F)
            stg1, agi1, ago1, slab1 = mk_round("1", OUT)
            stg2, agi2, ago2, slab2 = mk_round("2", OUT)

            def piece_of(mb):
                i = mb // 4
                return i, mb - (0, 4, 8)[i]

            def emit_round(stg_i, agi_i, ago_i, slab_i):
                nc.sync.dma_start(agi_i[:], stg_i[:])
                nc.gpsimd.collective_compute(
                    "AllGather", mybir.AluOpType.bypass,
                    replica_groups=[list(range(CORES))],
                    ins=[agi_i[:].opt()], outs=[ago_i[:].opt()],
                )
                # slab reload split per source core, in kp-consumption order,
                # so the first pairs are usable right after the CC completes.
                # Scalar queue so the AG-semaphore wait can't block staging
                # DMAs behind it on the in-order sync queue.
                for c8 in range(CORES):
                    nc.scalar.dma_start(
                        slab_i[:, c8:c8 + 1, :].rearrange("p c n -> p (c n)"),
                        ago_i[c8 * P:(c8 + 1) * P, :])

            def u_of(slabs, w):
                def f(kp):
                    c8, mp = divmod(kp, PPC)
                    i = 0 if mp < 2 else (1 if mp < 4 else 2)
                    j = mp - (0, 2, 4)[i]
                    return slabs[i][:, c8:c8 + 1, 2 * j * w:(2 * j + 2) * w] \
                        .rearrange("p c (k n) -> p (c k) n", k=2)
                return f

            uB0 = u_of(slab0, F)    # hop B moving operand (s1)
            uA1 = u_of(slab1, OUT)  # hop C stationary (dis*z2)
            uB2 = u_of(slab2, OUT)  # hop D stationary (sD)

            def stage_s1(mb, pp_ap):
                i, idx = piece_of(mb)
                t = wrk.tile([P, F], BF16, tag="st", bufs=2, name=f"stA_{mb}")
                nc.vector.tensor_scalar_mul(t[:], pp_ap, dd[:, mb:mb + 1])
                nc.vector.tensor_add(stg0[i][:, idx * F:(idx + 1) * F],
                                     t[:], d1z_t[mb][:])

            for mb in range(MB):
                stage_s1(mb, ppA[mb])
                if mb in (3, 7, 9):
                    i = piece_of(mb)[0]
                    emit_round(stg0[i], agi0[i], ago0[i], slab0[i])

            # ---- hop B + layer-2 feature matmuls, software-pipelined ----
            # kp consumption order matches round-0 piece arrival
            orderB = [c8 * PPC + mp for ms in PCP for mp in ms
                      for c8 in range(CORES)]

            pp_b = [None] * MB

            def post_B(m):
                h = wrk.tile([P, F], BF16, tag="h", bufs=2, name=f"h_{m}")
                nc.vector.tensor_scalar_mul(h[:], pp_b[m][:], ndiso[:, m:m + 1])
                nc.vector.tensor_add(h[:], h[:], e0_t[m][:])
                nc.vector.tensor_add(h[:], h[:], b1r[:])
                nc.vector.tensor_scalar_max(h[:], h[:], 0.0)
                hT = []
                for c in range(2):
                    tps = ptr.tile([P, P], BF16, tag="tr", name=f"hTp_{m}_{c}")
                    nc.tensor.transpose(tps[:], h[:, c * P:(c + 1) * P], idb[:])
                    tb = res.tile([P, HID], BF16, tag="wh", bufs=6, name=f"hTs_{m}_{c}")
                    nc.vector.tensor_copy(tb[:, :P], tps[:])
                    hT.append(tb[:, :P])
                # all three h@W2k products in one psum, hT stationary loaded
                # once per c-half (3 consecutive matmuls share the LDWEIGHTS)
                zall = pterm.tile([P, 3 * OUT], F32, tag="tp", name=f"zall_{m}")
                for c in range(2):
                    for j, k in enumerate((1, 2, 0)):  # z1 | z2 | hw
                        # single bank-zero on the first matmul (see hop A note)
                        nc.tensor.matmul(zall[:, j * OUT:(j + 1) * OUT],
                                         hT[c], w2t[k][c][:],
                                         start=(c == 0 and j == 0), stop=(c == 1),
                                         skip_group_check=True)
                nc.vector.tensor_scalar_mul(d1z_t[m][:, :OUT], zall[:, :OUT],
                                            diso[:, m:m + 1])
                i, idx = piece_of(m)
                nc.vector.tensor_scalar_mul(stg1[i][:, idx * OUT:(idx + 1) * OUT],
                                            zall[:, OUT:2 * OUT], diso[:, m:m + 1])
                nc.vector.tensor_copy(hw_t[m][:, :OUT], zall[:, 2 * OUT:3 * OUT])
                if m in (3, 7, 9):
                    emit_round(stg1[i], agi1[i], ago1[i], slab1[i])

            for mb in range(5):
                pp_b[mb] = pprop.tile([P, F], F32, tag="pp", name=f"ppb_{mb}")
            for kp in orderB:
                for mb in range(5):
                    nc.tensor.matmul(
                        pp_b[mb][:], at_pair(kp, mb * P, P), uB0(kp),
                        start=(kp == orderB[0]), stop=(kp == orderB[-1]),
                        perf_mode=DR,
                    )
            for mb in range(5):
                post_B(mb)
            for mb in range(5, MB + 1):
                if mb < MB:
                    pp_b[mb] = pprop.tile([P, F], F32, tag="pp", name=f"ppb_{mb}")
                    for kp in orderB:
                        nc.tensor.matmul(
                            pp_b[mb][:], at_pair(kp, mb * P, P), uB0(kp),
                            start=(kp == orderB[0]), stop=(kp == orderB[-1]),
                            perf_mode=DR,
                        )
                if mb > 5:
                    post_B(mb - 1)

            # ---- hops C and D: transposed (stationary = u pair tile, moving
            # = AT), psum j covers dst cols [j*256, (j+1)*256) = mbs (2j,2j+1).
            # Hop C runs in two sub-sweeps so R2 pieces stream out early.
            # kp order matches the 3-piece arrival of the previous round.
            orderCD = orderB

            ppc = [None] * PPC

            def sweep_T(u_fn, js, order, tagn):
                for j in js:
                    ppc[j] = pprop.tile([P, 2 * P], F32, tag="pp",
                                        name=f"{tagn}_{j}")
                for kp in order:
                    for j in js:
                        nc.tensor.matmul(
                            ppc[j][:], u_fn(kp),
                            at_pair(kp, j * 2 * P, 2 * P),
                            start=(kp == order[0]), stop=(kp == order[-1]),
                            perf_mode=DR,
                        )

            def post_C(j):
                sT = res.tile([P, 2 * P], F32, tag="xoTsT", bufs=2, name=f"sTc_{j}")
                nc.vector.tensor_copy(sT[:], ppc[j][:])
                for i in range(2):
                    mb = 2 * j + i
                    tps = ptr.tile([P, P], F32, tag="tr", name=f"trC_{mb}")
                    nc.tensor.transpose(tps[:], sT[:, i * P:(i + 1) * P], ident[:])
                    t = wrk.tile([P, F], BF16, tag="st", bufs=2, name=f"stC_{mb}")
                    nc.vector.tensor_scalar_mul(t[:, :OUT], tps[:], dd[:, mb:mb + 1])
                    i2, idx = piece_of(mb)
                    nc.vector.tensor_add(stg2[i2][:, idx * OUT:(idx + 1) * OUT],
                                         t[:, :OUT], d1z_t[mb][:, :OUT])
                    if mb in (3, 7, 9):
                        emit_round(stg2[i2], agi2[i2], ago2[i2], slab2[i2])

            # one sub-sweep per psum so each post_C (and its R2 piece) overlaps
            # the next sub-sweep's matmuls
            for j in range(PPC):
                sweep_T(uA1, (j,), orderCD, f"ppc{j}")
                post_C(j)

            def post_D(j):
                sT = res.tile([P, 2 * P], F32, tag="xoTsT", bufs=2, name=f"sTd_{j}")
                nc.vector.tensor_copy(sT[:], ppc[j][:])
                for i in range(2):
                    mb = 2 * j + i
                    tps = ptr.tile([P, P], F32, tag="tr", name=f"trD_{mb}")
                    nc.tensor.transpose(tps[:], sT[:, i * P:(i + 1) * P], ident[:])
                    oacc = wrk.tile([P, OUT], F32, tag="sf", bufs=2, name=f"oacc_{mb}")
                    nc.vector.tensor_scalar_mul(oacc[:], tps[:], ndiso[:, mb:mb + 1])
                    nc.vector.tensor_add(oacc[:], oacc[:], hw_t[mb][:, :OUT])
                    nc.vector.tensor_add(oacc[:], oacc[:], b2r[:])
                    nc.sync.dma_start(out_d[mb * P:(mb + 1) * P, :], oacc[:])

            # hop D in two sub-sweeps so the first output rows DMA out while
            # the second sub-sweep still accumulates
            # one sub-sweep per psum so each post_D (output rows) overlaps the
            # next sub-sweep's matmuls
            for j in range(PPC):
                sweep_T(uB2, (j,), orderCD, f"ppd{j}")
                post_D(j)

    nc.compile()
    return nc


def _prepare_inputs(x, edge, W1, b1, W2, b2):
    x = np.asarray(x, np.float32)
    edge = np.asarray(edge)
    W1 = np.asarray(W1, np.float32)
    b1 = np.asarray(b1, np.float32)
    W2 = np.asarray(W2, np.float32)
    b2 = np.asarray(b2, np.float32)
    src = edge[0].astype(np.int64)
    dst = edge[1].astype(np.int64)

    deg = np.bincount(dst, minlength=N).astype(np.float32)
    dis = np.where(deg > 0, 1.0 / np.sqrt(np.maximum(deg, 1.0)), 0.0).astype(np.float32)

    # dense transposed adjacency counts AT[s, d]
    flat = src * NP + dst
    uniq, cnt = np.unique(flat, return_counts=True)
    at8 = np.zeros(NP * NP, dtype=ml_dtypes.float8_e4m3)
    at8[uniq] = cnt.astype(ml_dtypes.float8_e4m3)
    at8 = at8.reshape(NP, NP)

    dis_pad = np.zeros(NP, np.float32)
    dis_pad[:N] = dis
    x_pad = np.zeros((NP, F), np.float32)
    x_pad[:N] = x

    w1x = np.stack([W1[0] - W1[2], W1[1], 2.0 * W1[2]]).astype(ml_dtypes.bfloat16)
    w2x = np.stack([W2[0] - W2[2], W2[1], 2.0 * W2[2]]).astype(ml_dtypes.bfloat16)
    b1r = np.broadcast_to(b1, (P, HID)).astype(ml_dtypes.bfloat16).copy()
    b2r = np.broadcast_to(b2, (P, OUT)).astype(ml_dtypes.bfloat16).copy()

    # x^T fp8, chunk-major with IN-halves pair-interleaved: [P, HCH, 2, HWC]
    xT8 = np.ascontiguousarray(
        x_pad.T.astype(ml_dtypes.float8_e4m3)
        .reshape(2, P, HCH, HWC).transpose(1, 2, 0, 3))
    w12p = np.ascontiguousarray(
        (2.0 * W1[2]).astype(ml_dtypes.float8_e4m3)
        .reshape(2, P, HID).transpose(1, 0, 2))
    disf_h = np.ascontiguousarray(dis_pad.reshape(KT, P).T)
    in_maps = []
    for c in range(CORES):
        rows = slice(c * RPC, (c + 1) * RPC)
        dv = dis_pad[rows]
        atc = np.ascontiguousarray(
            at8[:, rows].reshape(KT, P, RPC).transpose(1, 0, 2).reshape(P, KT * RPC))
        m = {
            "at": atc,
            "xoT": np.ascontiguousarray(x_pad[rows].T).astype(ml_dtypes.bfloat16),
            "xT8": xT8,
            "w12p": w12p,
            "disf": disf_h,
            "diso": np.ascontiguousarray(dv.reshape(MB, P).T),
            "ndiso": np.ascontiguousarray((-dv).reshape(MB, P).T),
            "dd": np.ascontiguousarray((-dv * dv).reshape(MB, P).T),
            "w1x": w1x,
            "w2x": w2x,
            "b1r": b1r,
            "b2r": b2r,
        }
        in_maps.append(m)
    return in_maps


def _run(in_maps, trace=False, **kw):
    if "nc" not in _STATE:
        _STATE["nc"] = _build()
    r = run_bass_kernel_spmd(_STATE["nc"], in_maps, core_ids=list(range(CORES)),
                             trace=trace, **kw)
    out = np.concatenate([r.results[c]["outo"] for c in range(CORES)], axis=0)
    return out[:N], r


def kernel(**inputs) -> np.ndarray:
    in_maps = _prepare_inputs(**inputs)
    out, _ = _run(in_maps)
    return out
